# revision 24
# baseline (speedup 1.0000x reference)
"""Trainium2 Bass kernel for nn_CausalTemporalAttention.

Reference semantics (B == L == H == 8 required by the module's broadcast quirks):
  qkv = x @ w_qkv.T + b_qkv ; split q,k,v -> [B,L,H,S,d]
  scores[b,l,h,s,t] = q.k/sqrt(d) ; masked to -1e9 where h > l
  z = scores * decay_params[b,l,h] ; attn = softmax over l (the layer axis)
  out[b,l,h,s,:] = attn @ v ; swap (l,h) ; row-major reshape to [B*H, S, E]
  y = out @ w_out.T + b_out ; reshape [B,L,S,E]

Sharding: data-parallel over batch B across 8 cores (core i handles b=i).
All matmuls run in bf16 with fp32 PSUM accumulation; softmax in fp32.
Masked (l,h) pairs (h > l) produce exactly-zero attention rows and are
skipped everywhere (QKV columns, attention, and fully-zero out-proj tiles).
"""

import os
import sys

import numpy as np
import ml_dtypes

if "/opt/trn_rl_repo" not in sys.path:
    sys.path.insert(0, "/opt/trn_rl_repo")

B, L, S, E = 8, 8, 256, 1024
H, d = 8, E // 8
T = L * S            # 2048 tokens per batch element
NE = E // 128        # 8 e-chunks
F = 3 * E

# (l, h) pairs with h <= l, ordered by h (the attention processing order)
PAIRS = [(l, h) for h in range(H) for l in range(h, L)]
BLK = {p: i for i, p in enumerate(PAIRS)}  # 36 blocks

_BUILD_CACHE = {}


def _build(with_bias):
    import concourse.bass as bass
    import concourse.tile as tile
    import concourse.mybir as mybir
    from concourse import bacc
    from contextlib import ExitStack

    dt = mybir.dt
    AF = mybir.ActivationFunctionType

    nc = bacc.Bacc("TRN2", target_bir_lowering=False, debug=False, num_devices=8)

    xT_d = nc.dram_tensor("xT", [E, T], dt.bfloat16, kind="ExternalInput").ap()
    # q/k weights pre-packed on host as [part, head, p, e*128+m] so each
    # (part, head) loads with ONE contiguous-per-partition DMA.
    wqkp_d = nc.dram_tensor("wqkp", [2, H, 128, E], dt.bfloat16, kind="ExternalInput").ap()
    wv_d = nc.dram_tensor("wvT", [E, E], dt.bfloat16, kind="ExternalInput").ap()
    wo_d = nc.dram_tensor("woutT", [E, E], dt.bfloat16, kind="ExternalInput").ap()
    dec_d = nc.dram_tensor("decay", [128, L * H], dt.float32, kind="ExternalInput").ap()
    if with_bias:
        bq_d = nc.dram_tensor("bqkv", [1, F], dt.bfloat16, kind="ExternalInput").ap()
        bo_d = nc.dram_tensor("bout", [1, E], dt.bfloat16, kind="ExternalInput").ap()
        bor_d = nc.dram_tensor("bout_row", [128, E], dt.float32, kind="ExternalInput").ap()
    y_d = nc.dram_tensor("y", [H, S, E], dt.float32, kind="ExternalOutput").ap()

    with ExitStack() as ctx:
        ctx.enter_context(
            nc.allow_low_precision(
                reason="bf16 softmax intermediates; end-to-end error ~3e-3 of scale"
            )
        )
        tc = ctx.enter_context(tile.TileContext(nc))

        consts = ctx.enter_context(tc.tile_pool(name="consts", bufs=1))
        # One tile + one DMA writer per chunk: a tile written by DMAs spread
        # over many HW queues gives its first consumer more sync waits than
        # the MM instruction can encode ("Too many sync wait commands"), and
        # fine-grained tiles let compute start as soon as its chunk lands.
        xT_sb = [consts.tile([128, T], dt.bfloat16, name=f"xT{e}") for e in range(NE)]
        wqv_sb = [consts.tile([128, E], dt.bfloat16, name=f"wqv{e}") for e in range(NE)]
        # q/k weights: one tile per (part, head) holding all e-chunks
        # side-by-side, loaded head-major so head 0's tiles land first.
        wqk_sb = {
            (part, h): consts.tile([128, E], dt.bfloat16, name=f"w{part}{h}")
            for part in ("q", "k")
            for h in range(H)
        }
        wo_sb = [consts.tile([128, E], dt.bfloat16, name=f"wo{e}") for e in range(NE)]
        dec_sb = consts.tile([128, L * H], dt.float32)
        v_sb = consts.tile([128, len(PAIRS), 2, d], dt.bfloat16)
        zrow_sb = consts.tile([128, 512], dt.float32)

        if with_bias:
            bq_sb = consts.tile([1, F], dt.bfloat16)
            bo_sb = consts.tile([1, E], dt.bfloat16)
            ones_sb = consts.tile([1, 512], dt.bfloat16)
            borow_sb = consts.tile([128, E], dt.float32)
            nc.sync.dma_start(out=bq_sb, in_=bq_d)
            nc.sync.dma_start(out=bo_sb, in_=bo_d)
            nc.sync.dma_start(out=borow_sb, in_=bor_d)
            nc.vector.memset(ones_sb, 1.0)

        nc.vector.memset(zrow_sb, 0.0)
        nc.sync.dma_start(out=dec_sb, in_=dec_d)
        # Three DMA queues in parallel, each loading in consumption order:
        #   SP:    xT chunks (feed everything, needed from the first MM)
        #   ACT:   q/k weight tiles, head-major (head 0 first)
        #   Pool:  v weights (needed ~15us in), then out-proj weights (~60us)
        # Each dma_start lands on ONE ~43GB/s HW ring, so split every tensor
        # into chunks issued in waves across the rings, in the exact order
        # compute consumes them (xT quarter q == layer-pair q of qk_proj).
        for q in range(4):
            for e in range(NE):
                nc.sync.dma_start(
                    out=xT_sb[e][:, q * 512:(q + 1) * 512],
                    in_=xT_d[e * 128:(e + 1) * 128, q * 512:(q + 1) * 512],
                )
        for g in range(2):
            for e in range(NE):
                nc.sync.dma_start(
                    out=wqv_sb[e][:, g * 512:(g + 1) * 512],
                    in_=wv_d[e * 128:(e + 1) * 128, g * 512:(g + 1) * 512],
                )
        for h in range(H):
            for pi, part in ((0, "q"), (1, "k")):
                for half in range(2):
                    nc.scalar.dma_start(
                        out=wqk_sb[(part, h)][:, half * 512:(half + 1) * 512],
                        in_=wqkp_d[pi, h, :, half * 512:(half + 1) * 512],
                    )
        for e in range(NE):
            for half in range(2):
                nc.scalar.dma_start(
                    out=wo_sb[e][:, half * 512:(half + 1) * 512],
                    in_=wo_d[e * 128:(e + 1) * 128, half * 512:(half + 1) * 512],
                )

        mm_ps = ctx.enter_context(tc.tile_pool(name="mm_ps", bufs=3, space="PSUM"))
        sc_ps = ctx.enter_context(tc.tile_pool(name="sc_ps", bufs=3, space="PSUM"))
        o2_ps = ctx.enter_context(tc.tile_pool(name="o2_ps", bufs=2, space="PSUM"))

        qk_pool = ctx.enter_context(tc.tile_pool(name="qk", bufs=2))
        exp_pool = ctx.enter_context(tc.tile_pool(name="expp", bufs=1))
        sm_pool = ctx.enter_context(tc.tile_pool(name="smp", bufs=2))
        at_pool = ctx.enter_context(tc.tile_pool(name="atp", bufs=3))
        gt_pool = ctx.enter_context(tc.tile_pool(name="gtp", bufs=2))
        out_pool = ctx.enter_context(tc.tile_pool(name="outp", bufs=3))

        def v_proj():
            # v projection (natural [token, dd] layout): stationary xT tile,
            # moving w columns. Only heads h <= l are ever read.
            for tt in range(T // 128):
                l = tt // 2
                ncols = 128 * (l + 1)
                for g in range((ncols + 511) // 512):
                    n_g = min(512, ncols - 512 * g)
                    p_v = mm_ps.tile([128, n_g], dt.float32, tag="mm", name="p_v")
                    for e in range(NE):
                        nc.tensor.matmul(
                            p_v,
                            lhsT=xT_sb[e][:, tt * 128:(tt + 1) * 128],
                            rhs=wqv_sb[e][:, 512 * g: 512 * g + n_g],
                            start=(e == 0),
                            stop=(e == NE - 1) and not with_bias,
                        )
                    if with_bias:
                        nc.tensor.matmul(
                            p_v,
                            lhsT=ones_sb[:, :128],
                            rhs=bq_sb[:, 2 * E + 512 * g: 2 * E + 512 * g + n_g],
                            start=False,
                            stop=True,
                        )
                    for hh in range(4 * g, 4 * g + n_g // 128):
                        nc.vector.tensor_copy(
                            out=v_sb[:, BLK[(l, hh)], tt % 2, :],
                            in_=p_v[:, (hh - 4 * g) * 128:(hh - 4 * g + 1) * 128],
                        )

        # ---- per-head pipeline: q/k projection -> scores -> softmax-over-l ->
        # attn@v -> scatter into the scrambled proj input -> out projection.
        # The q/k projection for head h+1 is emitted between head h's scores
        # and attn@v so the PE has work while the softmax chain (ACT+DVE) runs.
        def qk_proj(h):
            qT = qk_pool.tile([128, L, S], dt.bfloat16, tag="qT", name="qT_sb")
            kT = qk_pool.tile([128, L, S], dt.bfloat16, tag="kT", name="kT_sb")
            for part, base, dst in (("q", 0, qT), ("k", E, kT)):
                l = h
                while l < L:
                    nl = 2 if l + 1 < L else 1  # pair layers: N=512 moving dim
                    p_qk = mm_ps.tile([128, nl * S], dt.float32, tag="mm", name="p_qk")
                    for e in range(NE):
                        nc.tensor.matmul(
                            p_qk,
                            lhsT=wqk_sb[(part, h)][:, e * 128:(e + 1) * 128],
                            rhs=xT_sb[e][:, l * S:(l + nl) * S],
                            start=(e == 0),
                            stop=(e == NE - 1) and not with_bias,
                        )
                    if with_bias:
                        nc.tensor.matmul(
                            p_qk,
                            lhsT=bq_sb[:, base + h * 128: base + (h + 1) * 128],
                            rhs=ones_sb[:, :nl * S],
                            start=False,
                            stop=True,
                        )
                    src = p_qk.rearrange("p (a b) -> p a b", a=nl)
                    if part == "q":
                        nc.scalar.copy(out=dst[:, l:l + nl, :], in_=src)
                    else:
                        nc.vector.tensor_copy(out=dst[:, l:l + nl, :], in_=src)
                    l += nl
            return qT, kT

        qk_tiles = qk_proj(0)
        v_proj()
        for h in range(H):
            qT_sb, kT_sb = qk_tiles
            # scores (transposed [t, s]) + exp with decay/sqrt(d) folded into
            # the activation scale; D accumulates the softmax denominator.
            # Softmax intermediates in bf16: DVE runs 2-byte SBUF ops in the
            # fast perf modes, and the end-to-end error stays ~3e-3 of scale.
            E_sb = exp_pool.tile([128, L, 2, S], dt.bfloat16, tag="E", name="E_sb")
            D_sb = sm_pool.tile([128, 2, S], dt.bfloat16, tag="D", name="D_sb")
            for l in range(h, L):
                p_sc = sc_ps.tile([128, 2, S], dt.float32, tag="sc", name="p_sc")
                for tc2 in range(2):
                    nc.tensor.matmul(
                        p_sc[:, tc2, :],
                        lhsT=kT_sb[:, l, tc2 * 128:(tc2 + 1) * 128],
                        rhs=qT_sb[:, l, :],
                        start=True,
                        stop=True,
                    )
                idx = l * H + h
                nc.scalar.activation(
                    out=E_sb[:, l, :, :],
                    in_=p_sc,
                    func=AF.Exp,
                    scale=dec_sb[:, idx:idx + 1],
                )
                if l == h:
                    nc.vector.tensor_copy(out=D_sb, in_=E_sb[:, l, :, :])
                else:
                    nc.vector.tensor_add(D_sb, D_sb, E_sb[:, l, :, :])

            # next head's projection fills the PE while softmax finishes
            if h + 1 < H:
                qk_tiles = qk_proj(h + 1)

            U_sb = sm_pool.tile([128, 2, S], dt.bfloat16, tag="U", name="U_sb")
            nc.vector.reciprocal(out=U_sb, in_=D_sb)

            # attn @ v (output transposed [dd, s]) and scatter into GT, the
            # transposed input of the out-projection:
            #   GT[dd, j, l*32 + si] = out2T[dd, si*8 + j]
            gt_sb = gt_pool.tile([128, L, S], dt.bfloat16, tag="gt", name="gt_sb")
            if h > 0:
                nc.vector.memset(gt_sb[:, :, :h * 32], 0.0)
            for l in range(h, L):
                at_sb = at_pool.tile([128, 2, S], dt.bfloat16, tag="at", name="at_sb")
                nc.vector.tensor_mul(at_sb, E_sb[:, l, :, :], U_sb)
                p_o2 = o2_ps.tile([128, S], dt.float32, tag="o2", name="p_o2")
                for tc2 in range(2):
                    nc.tensor.matmul(
                        p_o2,
                        lhsT=v_sb[:, BLK[(l, h)], tc2, :],
                        rhs=at_sb[:, tc2, :],
                        start=(tc2 == 0),
                        stop=(tc2 == 1),
                    )
                nc.vector.tensor_copy(
                    out=gt_sb[:, :, l * 32:(l + 1) * 32],
                    in_=p_o2.rearrange("p (si j) -> p j si", j=8),
                )

            # out projection for this head: y[h, s', :] = GT.T @ woutT (+ b_out)
            for st in range(2):
                if st == 0 and h >= 4:
                    # rows s' in [0,128) are exactly zero for h >= 4
                    for ng in range(2):
                        src = borow_sb[:, ng * 512:(ng + 1) * 512] if with_bias else zrow_sb
                        nc.sync.dma_start(
                            out=y_d[h, :128, ng * 512:(ng + 1) * 512], in_=src
                        )
                    continue
                for ng in range(2):
                    p_pr = mm_ps.tile([128, 512], dt.float32, tag="mm", name="p_pr")
                    for j in range(NE):
                        nc.tensor.matmul(
                            p_pr,
                            lhsT=gt_sb[:, j, st * 128:(st + 1) * 128],
                            rhs=wo_sb[j][:, ng * 512:(ng + 1) * 512],
                            start=(j == 0),
                            stop=(j == NE - 1) and not with_bias,
                        )
                    if with_bias:
                        nc.tensor.matmul(
                            p_pr,
                            lhsT=ones_sb[:, :128],
                            rhs=bo_sb[:, ng * 512:(ng + 1) * 512],
                            start=False,
                            stop=True,
                        )
                    o_sb = out_pool.tile([128, 512], dt.float32, tag="o", name="o_sb")
                    nc.scalar.copy(out=o_sb, in_=p_pr)
                    nc.sync.dma_start(
                        out=y_d[h, st * 128:(st + 1) * 128, ng * 512:(ng + 1) * 512],
                        in_=o_sb,
                    )

    nc.compile()
    return nc


def _get_nc(with_bias):
    if with_bias not in _BUILD_CACHE:
        _BUILD_CACHE[with_bias] = _build(with_bias)
    return _BUILD_CACHE[with_bias]


def _prepare_in_maps(x, w_qkv, b_qkv, w_out, b_out, decay_params):
    bf16 = ml_dtypes.bfloat16
    with_bias = bool(np.any(b_qkv != 0) or np.any(b_out != 0))

    wqk_bf = w_qkv[:2 * E].astype(bf16)                          # [2E, E]
    # [part, head, m, e, p] -> [part, head, p, e, m]: each (part, head) tile
    # is the stationary lhsT for all e-chunks, contiguous in DRAM.
    wqkp = np.ascontiguousarray(
        wqk_bf.reshape(2, H, d, NE, 128).transpose(0, 1, 4, 3, 2)
    ).reshape(2, H, 128, E)
    wvT = np.ascontiguousarray(w_qkv[2 * E:].astype(bf16).T)     # [E, E]
    woutT = np.ascontiguousarray(w_out.astype(bf16).T)           # [E, E]

    in_maps = []
    for b in range(B):
        xT = np.ascontiguousarray(
            x[b].reshape(T, E).astype(bf16).T                    # [E, T]
        )
        dec = np.ascontiguousarray(
            np.broadcast_to(
                (decay_params[b, :L, :H] / np.float32(np.sqrt(d)))
                .astype(np.float32)
                .reshape(1, L * H),
                (128, L * H),
            )
        )
        m = {"xT": xT, "wqkp": wqkp, "wvT": wvT, "woutT": woutT, "decay": dec}
        if with_bias:
            m["bqkv"] = np.ascontiguousarray(b_qkv.astype(bf16).reshape(1, F))
            m["bout"] = np.ascontiguousarray(b_out.astype(bf16).reshape(1, E))
            m["bout_row"] = np.ascontiguousarray(
                np.broadcast_to(b_out.astype(np.float32).reshape(1, E), (128, E))
            )
        in_maps.append(m)
    return with_bias, in_maps


def _run(x, w_qkv, b_qkv, w_out, b_out, decay_params, **spmd_kwargs):
    from concourse.bass_utils import run_bass_kernel_spmd

    with_bias, in_maps = _prepare_in_maps(x, w_qkv, b_qkv, w_out, b_out, decay_params)
    nc = _get_nc(with_bias)
    res = run_bass_kernel_spmd(nc, in_maps, core_ids=list(range(B)), **spmd_kwargs)
    out = np.stack([r["y"] for r in res.results], axis=0)  # [B, H, S, E]
    return out.astype(np.float32, copy=False), res


def kernel(x, w_qkv, b_qkv, w_out, b_out, decay_params):
    out, _ = _run(
        np.asarray(x), np.asarray(w_qkv), np.asarray(b_qkv),
        np.asarray(w_out), np.asarray(b_out), np.asarray(decay_params),
    )
    return out
